# revision 30
# baseline (speedup 1.0000x reference)
"""ChebyASPIRE spectral filter on 8 TRN2 NeuronCores.

Algorithm (Gram-matrix formulation):
  phase 1: Z = X^T X  (4096x4096), column-sharded: core i computes
           Z[:, i*512:(i+1)*512] from a full stream of X in fp8e4m3
           using DoubleRow matmuls (2 k-rows packed per partition,
           2x PE throughput vs fp16), fp32 PSUM accumulate, Z kept in
           SBUF as fp16.
  phase 2: Chebyshev recurrence t_k = 2*Zs t_{k-1} - t_{k-2} with
           Zs = (Z - t_mid I)/t_half, applied to V = R^T (4096x256).
           Row-sharded: core i computes rows [i*512, (i+1)*512) of each
           t_k using lhsT = Z[:, ib] (== Z[ib, :]^T by symmetry), then
           AllGathers the new t shard in fp8 (half the collective bytes
           of fp16; the gathered t is upcast to fp16 on-chip for the
           matmul). The batch is split into two halves that alternate
           on the PE so each half's AllGather hides under the other
           half's matmuls. Recurrence state and accumulator stay fp32.

Inputs come in full; sharding/layout prep happens on host (X is
pre-quantized to fp8 and pre-packed into the DoubleRow SBUF tile layout
so all phase-1 DMAs are contiguous).  Scalars (t_mid, t_half, coeffs)
are baked into the program as immediates; the program is rebuilt (and
NEFF-cached) per distinct scalar set.
"""
import sys

sys.path.insert(0, "/opt/trn_rl_repo")

import numpy as np
import ml_dtypes

M, N, B = 8192, 4096, 256
NC = 8
CB = N // NC          # 512 columns/rows per core
DEG = 20              # Chebyshev degree (21 coeffs)
KT1 = M // 128        # 64 k-tiles in phase 1
KP1 = KT1 // 2        # 32 DoubleRow k-pair tiles in phase 1
MP1 = N // 128        # 32 m-passes in phase 1
KT2 = N // 128        # 32 k-tiles in phase 2
MS2 = CB // 128       # 4 m-subs in phase 2
NH = 2                # batch halves in phase 2
BH = B // NH          # 128 columns per half
LCH = 16              # k-pair tiles per lh chunk (phase 1): 2 chunks/pass
RCH = 4               # k-pair tiles per rhs chunk (phase 1): 8 chunks

F8NP = ml_dtypes.float8_e4m3   # numpy equivalent of mybir float8e4

_BUILD_CACHE = {}


def _build(scalars):
    """Build the SPMD Bass program for the given (t_mid, t_half, *coeffs)."""
    from concourse import bacc, tile, mybir

    tm, th = scalars[0], scalars[1]
    c = scalars[2:]
    f8 = mybir.dt.float8e4
    f16 = mybir.dt.float16
    f32 = mybir.dt.float32
    mult = mybir.AluOpType.mult
    add = mybir.AluOpType.add
    sub = mybir.AluOpType.subtract
    DR = mybir.MatmulPerfMode.DoubleRow

    nc = bacc.Bacc("TRN2", target_bir_lowering=False, debug=False,
                   num_devices=NC)
    # phase-1 fp8 stream, pre-packed: [p, mp, kpair, 2, mc]
    XL = nc.dram_tensor("XL8", [128, MP1, KP1, 2, 128], f8,
                        kind="ExternalInput")
    # phase-1 fp8 resident block: [p, kpair, 2, cb]
    XR = nc.dram_tensor("XR8", [128, KP1, 2, CB], f8, kind="ExternalInput")
    # V = R^T pre-packed p-major: [p, kk*B] with row r=kk*128+p at [p, kk, :]
    Vp = nc.dram_tensor("V16p", [128, KT2 * B], f16, kind="ExternalInput")
    Vb = nc.dram_tensor("Vblk32", [CB, B], f32, kind="ExternalInput")
    acc_out = nc.dram_tensor("acc_out", [CB, B], f32, kind="ExternalOutput")

    with tile.TileContext(nc) as tc:
        with (
            tc.tile_pool(name="persist", bufs=1) as persist,
            tc.tile_pool(name="lstream", bufs=4) as lstream,
            tc.tile_pool(name="rhsp8", bufs=2) as rhsp8,
            tc.tile_pool(name="rhsp16", bufs=2) as rhsp16,
            tc.tile_pool(name="dve", bufs=4) as dvep,
            tc.tile_pool(name="stagep", bufs=2) as stagep,
            tc.tile_pool(name="ps1", bufs=2, space="PSUM") as ps1,
            tc.tile_pool(name="ps2", bufs=6, space="PSUM") as ps2p,
            tc.tile_pool(name="dram", bufs=1, space="DRAM") as dram,
        ):
            # CC-path warmup: tiny AllGather concurrent with initial DMAs
            wsb = dvep.tile([1, 128], f8, name="wsb")
            nc.any.memset(wsb[:], 0.0)
            win = dram.tile([1, 128], f8, name="warm_in")
            nc.sync.dma_start(win[:, :], wsb[:, :])
            wout = dram.tile([NC, 128], f8, addr_space="Shared",
                             name="warm_out")
            nc.gpsimd.collective_compute(
                "AllGather", mybir.AluOpType.bypass,
                replica_groups=[list(range(NC))],
                ins=[win[:, :]], outs=[wout[:, :]])

            # ---------------- phase 1: Z[:, ib] = X^T X[:, ib] -------------
            # resident rhs X8[:, ib] in DR pack, chunked so matmuls start
            # early; first chunk is issued before the lh stream.
            rhs_res = [persist.tile([128, RCH, 2, CB], f8, name=f"rhs8_{cc}")
                       for cc in range(KP1 // RCH)]
            nc.sync.dma_start(rhs_res[0][:], XR[:, 0:RCH, :, :])

            zk = [persist.tile([128, CB], f16, name=f"zk{i}")
                  for i in range(KT2)]
            rh_s1 = []

            NLH = KP1 // LCH   # lh chunks per mp pass
            first_lh = [lstream.tile([128, LCH, 2, 128], f8,
                                     name=f"lh0_{h}") for h in range(NLH)]
            for h in range(NLH):
                nc.sync.dma_start(
                    first_lh[h][:],
                    XL[:, 0, h * LCH:(h + 1) * LCH, :, :])
            for cc in range(1, KP1 // RCH):
                nc.sync.dma_start(rhs_res[cc][:],
                                  XR[:, cc * RCH:(cc + 1) * RCH, :, :])

            for mp in range(MP1):
                if mp == 0:
                    lhs = first_lh
                else:
                    lhs = [lstream.tile([128, LCH, 2, 128], f8,
                                        name=f"lh_{h}") for h in range(NLH)]
                    for h in range(NLH):
                        nc.sync.dma_start(
                            lhs[h][:],
                            XL[:, mp, h * LCH:(h + 1) * LCH, :, :])
                zps = ps1.tile([128, CB], f32, name="zps")
                for kk in range(KP1):
                    nc.tensor.matmul(
                        zps[:],
                        lhs[kk // LCH][:, kk % LCH, :, :],
                        rhs_res[kk // RCH][:, kk % RCH, :, :],
                        start=(kk == 0), stop=(kk == KP1 - 1),
                        perf_mode=DR)
                nc.vector.tensor_copy(zk[mp][:], zps[:])
                if mp in (MP1 // 2, MP1 - 6):
                    # same-size warmup AllGathers mid-phase-1 and near the
                    # tail: they keep the RDH channel hot AND resync the
                    # cores so the first real AllGather absorbs less skew;
                    # dep on zk[mp] sequences them here
                    wz = stagep.tile([128, MS2 * BH], f8, name="wz")
                    nc.vector.tensor_copy(wz[:], zk[mp][:])
                    win2 = dram.tile([128, MS2 * BH], f8,
                                     name=f"warm_in2_{mp}")
                    nc.sync.dma_start(win2[:, :], wz[:, :])
                    wout2 = dram.tile([NC * 128, MS2 * BH], f8,
                                      addr_space="Shared",
                                      name=f"warm_out2_{mp}")
                    nc.gpsimd.collective_compute(
                        "AllGather", mybir.AluOpType.bypass,
                        replica_groups=[list(range(NC))],
                        ins=[win2[:, :]], outs=[wout2[:, :]])
                # step-1 rhs (fp16 halves of V): issued mid-phase so the
                # transfers don't compete with phase-1 startup DMAs (offset
                # from the warmup-AG insertion to avoid stacking bursts)
                if mp == MP1 // 2 + 2:
                    Vp4 = Vp[:, :].rearrange("p (i ms b) -> p i ms b",
                                             i=NC, ms=MS2)
                    for h in range(NH):
                        rh1 = rhsp16.tile([128, NC, MS2, BH], f16,
                                          name="rh")
                        for ch in range(4):
                            nc.sync.dma_start(
                                rh1[:, 2 * ch:2 * ch + 2, :, :],
                                Vp4[:, 2 * ch:2 * ch + 2, :,
                                    h * BH:(h + 1) * BH])
                        rh_s1.append(rh1)

            # ---------------- phase 2: Chebyshev recurrence ----------------
            # per-half fp32 state shards (128 part x [4 m-subs x 128])
            tstate = [[persist.tile([128, MS2, BH], f32, name=f"tst{h}_{i}")
                       for i in range(3)] for h in range(NH)]
            acc = [persist.tile([128, MS2, BH], f32, name=f"acc{h}")
                   for h in range(NH)]
            zero = persist.tile([128, BH], f32, name="zero")
            nc.any.memset(zero[:], 0.0)
            Vb3 = Vb[:, :].rearrange("(ms p) b -> p ms b", p=128)
            for h in range(NH):
                nc.sync.dma_start(tstate[h][0][:],
                                  Vb3[:, :, h * BH:(h + 1) * BH])

            agout = [[None] * NH for _ in range(DEG)]

            for s in range(1, DEG + 1):
                for h in range(NH):
                    # rhs: full t_{s-1} half (4096 x 128), k-packed.
                    # s=1: prefetched fp16 V half.  s>1: gathered fp8
                    # shards, p-major, used directly as the (mixed-dtype)
                    # matmul moving operand.  The loads are issued from the
                    # TENSOR engine right before this half's matmul burst:
                    # by then this half's AllGather (which ran concurrently
                    # with the other half's matmuls) is complete, so the PE
                    # barely waits, and the sync queue stays free for the
                    # stage DMAs.
                    if s == 1:
                        rh = rh_s1[h]
                    else:
                        rh = rhsp8.tile([128, NC, MS2, BH], f8, name="rh8")
                        src = (agout[s - 2][h][:, :]
                               .rearrange("(i p) (ms b) -> p i ms b",
                                          p=128, ms=MS2))
                        # chunk0 (shards 0-3, needed first) on the
                        # dedicated scalar engine; chunk1 on sync, whose
                        # mild head-block tolerance matches its later use
                        nc.scalar.dma_start(rh[:, 0:4, :, :],
                                            src[:, 0:4, :, :])
                        nc.sync.dma_start(rh[:, 4:8, :, :],
                                          src[:, 4:8, :, :])

                    Tc = tstate[h][(s - 1) % 3]
                    Tp = tstate[h][(s - 2) % 3] if s >= 2 else None
                    Tn = tstate[h][s % 3]
                    ach = acc[h]
                    if s < DEG:
                        stage = stagep.tile([128, MS2, BH], f8,
                                            name=f"stage{h}")
                        # p-major AG payload: agin[p, :] = stage partition p
                        agin = dram.tile([128, MS2 * BH], f8,
                                         name=f"agin{s}_{h}")
                        agin3 = agin[:, :].rearrange("p (ms b) -> p ms b",
                                                     ms=MS2)

                    for ms in range(MS2):
                        wps = ps2p.tile([128, BH], f32, name="wps")
                        for kk in range(KT2):
                            nc.tensor.matmul(
                                wps[:],
                                zk[kk][:, ms * 128:(ms + 1) * 128],
                                rh[:, kk // MS2, kk % MS2, :],
                                start=(kk == 0), stop=(kk == KT2 - 1))

                        u = dvep.tile([128, BH], f32, name="u")
                        # u = W - tm * Tc
                        nc.vector.scalar_tensor_tensor(
                            u[:], Tc[:, ms, :], -tm, wps[:],
                            op0=mult, op1=add)
                        if s == 1:
                            # T1 = u / th ;  acc = c0*V + c1*T1
                            nc.vector.scalar_tensor_tensor(
                                Tn[:, ms, :], u[:], 1.0 / th, zero[:],
                                op0=mult, op1=sub)
                        else:
                            # Tn = (2/th)*u - Tp
                            nc.vector.scalar_tensor_tensor(
                                Tn[:, ms, :], u[:], 2.0 / th, Tp[:, ms, :],
                                op0=mult, op1=sub)
                        if s < DEG:
                            nc.vector.tensor_copy(stage[:, ms, :],
                                                  Tn[:, ms, :])
                        # acc updates after the stage cast so the AG input
                        # is produced as early as possible
                        if s == 1:
                            nc.vector.tensor_scalar_mul(
                                ach[:, ms, :], Tc[:, ms, :], c[0])
                            nc.vector.scalar_tensor_tensor(
                                ach[:, ms, :], Tn[:, ms, :], c[1],
                                ach[:, ms, :], op0=mult, op1=add)
                        else:
                            nc.vector.scalar_tensor_tensor(
                                ach[:, ms, :], Tn[:, ms, :], c[s],
                                ach[:, ms, :], op0=mult, op1=add)

                    if s < DEG:
                        nc.sync.dma_start(agin3[:], stage[:])
                        agout[s - 1][h] = dram.tile(
                            [NC * 128, MS2 * BH], f8, addr_space="Shared",
                            name=f"agout{s}_{h}")
                        nc.gpsimd.collective_compute(
                            "AllGather",
                            mybir.AluOpType.bypass,
                            replica_groups=[list(range(NC))],
                            ins=[agin[:]],
                            outs=[agout[s - 1][h][:]],
                        )

            out3 = acc_out[:, :].rearrange("(ms p) b -> p ms b", p=128)
            for h in range(NH):
                nc.sync.dma_start(out3[:, :, h * BH:(h + 1) * BH],
                                  acc[h][:])

    nc.finalize()
    return nc


def _get_program(scalars):
    key = tuple(np.asarray(scalars, np.float64).tolist())
    if key not in _BUILD_CACHE:
        _BUILD_CACHE[key] = _build(key)
    return _BUILD_CACHE[key]


def _pack_inputs(X8):
    """Pre-pack fp8 X into the DoubleRow SBUF tile layouts (contiguous
    per-partition DMA lines)."""
    # stream: [p, mp, kpair, 2, mc] <- X8[(kpair*2+i)*128+p, mp*128+mc]
    XL = np.ascontiguousarray(
        X8.reshape(KP1, 2, 128, MP1, 128).transpose(2, 3, 0, 1, 4))
    return XL


def _run(X, R, coeffs, t_mid, t_half, trace=False):
    from concourse.bass_utils import run_bass_kernel_spmd

    X = np.ascontiguousarray(np.asarray(X, np.float32))
    R = np.ascontiguousarray(np.asarray(R, np.float32))
    coeffs = np.asarray(coeffs, np.float32)
    tm = float(np.asarray(t_mid).reshape(-1)[0])
    th = float(np.asarray(t_half).reshape(-1)[0])

    nc = _get_program((tm, th, *[float(v) for v in coeffs]))

    X8 = X.astype(F8NP)
    XL = _pack_inputs(X8)
    V32 = np.ascontiguousarray(R.T.astype(np.float32))   # (N, B)
    V16p = np.ascontiguousarray(
        V32.astype(np.float16).reshape(KT2, 128, B)
        .transpose(1, 0, 2).reshape(128, KT2 * B))

    in_maps = []
    for i in range(NC):
        ib = slice(i * CB, (i + 1) * CB)
        XRi = np.ascontiguousarray(
            X8[:, ib].reshape(KP1, 2, 128, CB).transpose(2, 0, 1, 3))
        in_maps.append({
            "XL8": XL,
            "XR8": XRi,
            "V16p": V16p,
            "Vblk32": np.ascontiguousarray(V32[ib, :]),
        })

    res = run_bass_kernel_spmd(nc, in_maps, core_ids=list(range(NC)),
                               trace=trace)

    out = np.empty((B, N), np.float32)
    for i in range(NC):
        out[:, i * CB:(i + 1) * CB] = res.results[i]["acc_out"].T
    return out, res


def kernel(X, R, coeffs, t_mid, t_half):
    out, _ = _run(X, R, coeffs, t_mid, t_half, trace=False)
    return out


# revision 31
# speedup vs baseline: 1.0358x; 1.0358x over previous
"""ChebyASPIRE spectral filter on 8 TRN2 NeuronCores.

Algorithm (Gram-matrix formulation):
  phase 1: Z = X^T X  (4096x4096), column-sharded: core i computes
           Z[:, i*512:(i+1)*512] from a full stream of X in fp8e4m3
           using DoubleRow matmuls (2 k-rows packed per partition,
           2x PE throughput vs fp16), fp32 PSUM accumulate, Z kept in
           SBUF as fp16.
  phase 2: Chebyshev recurrence t_k = 2*Zs t_{k-1} - t_{k-2} with
           Zs = (Z - t_mid I)/t_half, applied to V = R^T (4096x256).
           Row-sharded: core i computes rows [i*512, (i+1)*512) of each
           t_k using lhsT = Z[:, ib] (== Z[ib, :]^T by symmetry), then
           AllGathers the new t shard in fp8 (half the collective bytes
           of fp16; the gathered t is upcast to fp16 on-chip for the
           matmul). The batch is split into two halves that alternate
           on the PE so each half's AllGather hides under the other
           half's matmuls. Recurrence state and accumulator stay fp32.

Inputs come in full; sharding/layout prep happens on host (X is
pre-quantized to fp8 and pre-packed into the DoubleRow SBUF tile layout
so all phase-1 DMAs are contiguous).  Scalars (t_mid, t_half, coeffs)
are baked into the program as immediates; the program is rebuilt (and
NEFF-cached) per distinct scalar set.
"""
import sys

sys.path.insert(0, "/opt/trn_rl_repo")

import numpy as np
import ml_dtypes

M, N, B = 8192, 4096, 256
NC = 8
CB = N // NC          # 512 columns/rows per core
DEG = 20              # Chebyshev degree (21 coeffs)
KT1 = M // 128        # 64 k-tiles in phase 1
KP1 = KT1 // 2        # 32 DoubleRow k-pair tiles in phase 1
MP1 = N // 128        # 32 m-passes in phase 1
KT2 = N // 128        # 32 k-tiles in phase 2
MS2 = CB // 128       # 4 m-subs in phase 2
NH = 2                # batch halves in phase 2
BH = B // NH          # 128 columns per half
LCH = 16              # k-pair tiles per lh chunk (phase 1): 2 chunks/pass
RCH = 4               # k-pair tiles per rhs chunk (phase 1): 8 chunks

F8NP = ml_dtypes.float8_e4m3   # numpy equivalent of mybir float8e4

_BUILD_CACHE = {}


def _build(scalars):
    """Build the SPMD Bass program for the given (t_mid, t_half, *coeffs)."""
    from concourse import bacc, tile, mybir

    tm, th = scalars[0], scalars[1]
    c = scalars[2:]
    f8 = mybir.dt.float8e4
    f16 = mybir.dt.float16
    f32 = mybir.dt.float32
    mult = mybir.AluOpType.mult
    add = mybir.AluOpType.add
    sub = mybir.AluOpType.subtract
    DR = mybir.MatmulPerfMode.DoubleRow

    nc = bacc.Bacc("TRN2", target_bir_lowering=False, debug=False,
                   num_devices=NC)
    # phase-1 fp8 stream, pre-packed: [p, mp, kpair, 2, mc]
    XL = nc.dram_tensor("XL8", [128, MP1, KP1, 2, 128], f8,
                        kind="ExternalInput")
    # phase-1 fp8 resident block: [p, kpair, 2, cb]
    XR = nc.dram_tensor("XR8", [128, KP1, 2, CB], f8, kind="ExternalInput")
    # V = R^T pre-packed p-major: [p, kk*B] with row r=kk*128+p at [p, kk, :]
    Vp = nc.dram_tensor("V16p", [128, KT2 * B], f16, kind="ExternalInput")
    Vb = nc.dram_tensor("Vblk32", [CB, B], f32, kind="ExternalInput")
    acc_out = nc.dram_tensor("acc_out", [CB, B], f32, kind="ExternalOutput")

    with tile.TileContext(nc) as tc:
        with (
            tc.tile_pool(name="persist", bufs=1) as persist,
            tc.tile_pool(name="lstream", bufs=4) as lstream,
            tc.tile_pool(name="rhsp8", bufs=2) as rhsp8,
            tc.tile_pool(name="rhsp16", bufs=2) as rhsp16,
            tc.tile_pool(name="dve", bufs=4) as dvep,
            tc.tile_pool(name="stagep", bufs=2) as stagep,
            tc.tile_pool(name="ps1", bufs=2, space="PSUM") as ps1,
            tc.tile_pool(name="ps2", bufs=6, space="PSUM") as ps2p,
            tc.tile_pool(name="dram", bufs=1, space="DRAM") as dram,
        ):
            # CC-path warmup: tiny AllGather concurrent with initial DMAs
            wsb = dvep.tile([1, 128], f8, name="wsb")
            nc.any.memset(wsb[:], 0.0)
            win = dram.tile([1, 128], f8, name="warm_in")
            nc.sync.dma_start(win[:, :], wsb[:, :])
            wout = dram.tile([NC, 128], f8, addr_space="Shared",
                             name="warm_out")
            nc.gpsimd.collective_compute(
                "AllGather", mybir.AluOpType.bypass,
                replica_groups=[list(range(NC))],
                ins=[win[:, :]], outs=[wout[:, :]])

            # ---------------- phase 1: Z[:, ib] = X^T X[:, ib] -------------
            # resident rhs X8[:, ib] in DR pack, chunked so matmuls start
            # early; first chunk is issued before the lh stream.
            rhs_res = [persist.tile([128, RCH, 2, CB], f8, name=f"rhs8_{cc}")
                       for cc in range(KP1 // RCH)]
            nc.sync.dma_start(rhs_res[0][:], XR[:, 0:RCH, :, :])

            zk = [persist.tile([128, CB], f16, name=f"zk{i}")
                  for i in range(KT2)]
            rh_s1 = []

            NLH = KP1 // LCH   # lh chunks per mp pass
            first_lh = [lstream.tile([128, LCH, 2, 128], f8,
                                     name=f"lh0_{h}") for h in range(NLH)]
            for h in range(NLH):
                nc.sync.dma_start(
                    first_lh[h][:],
                    XL[:, 0, h * LCH:(h + 1) * LCH, :, :])
            for cc in range(1, KP1 // RCH):
                nc.sync.dma_start(rhs_res[cc][:],
                                  XR[:, cc * RCH:(cc + 1) * RCH, :, :])

            for mp in range(MP1):
                if mp == 0:
                    lhs = first_lh
                else:
                    lhs = [lstream.tile([128, LCH, 2, 128], f8,
                                        name=f"lh_{h}") for h in range(NLH)]
                    for h in range(NLH):
                        nc.sync.dma_start(
                            lhs[h][:],
                            XL[:, mp, h * LCH:(h + 1) * LCH, :, :])
                zps = ps1.tile([128, CB], f32, name="zps")
                for kk in range(KP1):
                    nc.tensor.matmul(
                        zps[:],
                        lhs[kk // LCH][:, kk % LCH, :, :],
                        rhs_res[kk // RCH][:, kk % RCH, :, :],
                        start=(kk == 0), stop=(kk == KP1 - 1),
                        perf_mode=DR)
                nc.vector.tensor_copy(zk[mp][:], zps[:])
                if mp in (MP1 // 2, MP1 - 6):
                    # same-size warmup AllGathers mid-phase-1 and near the
                    # tail: they keep the RDH channel hot AND resync the
                    # cores so the first real AllGather absorbs less skew;
                    # dep on zk[mp] sequences them here
                    wz = stagep.tile([128, MS2 * BH], f8, name="wz")
                    nc.vector.tensor_copy(wz[:], zk[mp][:])
                    win2 = dram.tile([128, MS2 * BH], f8,
                                     name=f"warm_in2_{mp}")
                    nc.sync.dma_start(win2[:, :], wz[:, :])
                    wout2 = dram.tile([NC * 128, MS2 * BH], f8,
                                      addr_space="Shared",
                                      name=f"warm_out2_{mp}")
                    nc.gpsimd.collective_compute(
                        "AllGather", mybir.AluOpType.bypass,
                        replica_groups=[list(range(NC))],
                        ins=[win2[:, :]], outs=[wout2[:, :]])
                # step-1 rhs (fp16 halves of V): issued mid-phase so the
                # transfers don't compete with phase-1 startup DMAs (offset
                # from the warmup-AG insertion to avoid stacking bursts)
                if mp == MP1 // 2 + 2:
                    Vp4 = Vp[:, :].rearrange("p (i ms b) -> p i ms b",
                                             i=NC, ms=MS2)
                    for h in range(NH):
                        rh1 = rhsp16.tile([128, NC, MS2, BH], f16,
                                          name="rh")
                        for ch in range(4):
                            nc.sync.dma_start(
                                rh1[:, 2 * ch:2 * ch + 2, :, :],
                                Vp4[:, 2 * ch:2 * ch + 2, :,
                                    h * BH:(h + 1) * BH])
                        rh_s1.append(rh1)

            # ---------------- phase 2: Chebyshev recurrence ----------------
            # per-half fp32 state shards (128 part x [4 m-subs x 128])
            tstate = [[persist.tile([128, MS2, BH], f32, name=f"tst{h}_{i}")
                       for i in range(3)] for h in range(NH)]
            acc = [persist.tile([128, MS2, BH], f32, name=f"acc{h}")
                   for h in range(NH)]
            zero = persist.tile([128, BH], f32, name="zero")
            nc.any.memset(zero[:], 0.0)
            Vb3 = Vb[:, :].rearrange("(ms p) b -> p ms b", p=128)
            for h in range(NH):
                nc.sync.dma_start(tstate[h][0][:],
                                  Vb3[:, :, h * BH:(h + 1) * BH])

            agout = [[None] * NH for _ in range(DEG)]

            for s in range(1, DEG + 1):
                for h in range(NH):
                    # rhs: full t_{s-1} half (4096 x 128), k-packed.
                    # s=1: prefetched fp16 V half.  s>1: gathered fp8
                    # shards, p-major, used directly as the (mixed-dtype)
                    # matmul moving operand.  The loads are issued from the
                    # TENSOR engine right before this half's matmul burst:
                    # by then this half's AllGather (which ran concurrently
                    # with the other half's matmuls) is complete, so the PE
                    # barely waits, and the sync queue stays free for the
                    # stage DMAs.
                    if s == 1:
                        rh = rh_s1[h]
                    else:
                        rh = rhsp8.tile([128, NC, MS2, BH], f8, name="rh8")
                        src = (agout[s - 2][h][:, :]
                               .rearrange("(i p) (ms b) -> p i ms b",
                                          p=128, ms=MS2))
                        for ch in range(2):
                            nc.scalar.dma_start(
                                rh[:, 4 * ch:4 * ch + 4, :, :],
                                src[:, 4 * ch:4 * ch + 4, :, :])

                    Tc = tstate[h][(s - 1) % 3]
                    Tp = tstate[h][(s - 2) % 3] if s >= 2 else None
                    Tn = tstate[h][s % 3]
                    ach = acc[h]
                    if s < DEG:
                        stage = stagep.tile([128, MS2, BH], f8,
                                            name=f"stage{h}")
                        # p-major AG payload: agin[p, :] = stage partition p
                        agin = dram.tile([128, MS2 * BH], f8,
                                         name=f"agin{s}_{h}")
                        agin3 = agin[:, :].rearrange("p (ms b) -> p ms b",
                                                     ms=MS2)

                    for ms in range(MS2):
                        wps = ps2p.tile([128, BH], f32, name="wps")
                        for kk in range(KT2):
                            nc.tensor.matmul(
                                wps[:],
                                zk[kk][:, ms * 128:(ms + 1) * 128],
                                rh[:, kk // MS2, kk % MS2, :],
                                start=(kk == 0), stop=(kk == KT2 - 1))

                        u = dvep.tile([128, BH], f32, name="u")
                        # u = W - tm * Tc
                        nc.vector.scalar_tensor_tensor(
                            u[:], Tc[:, ms, :], -tm, wps[:],
                            op0=mult, op1=add)
                        if s == 1:
                            # T1 = u / th ;  acc = c0*V + c1*T1
                            nc.vector.scalar_tensor_tensor(
                                Tn[:, ms, :], u[:], 1.0 / th, zero[:],
                                op0=mult, op1=sub)
                        else:
                            # Tn = (2/th)*u - Tp
                            nc.vector.scalar_tensor_tensor(
                                Tn[:, ms, :], u[:], 2.0 / th, Tp[:, ms, :],
                                op0=mult, op1=sub)
                        if s < DEG:
                            nc.vector.tensor_copy(stage[:, ms, :],
                                                  Tn[:, ms, :])
                        # acc updates after the stage cast so the AG input
                        # is produced as early as possible
                        if s == 1:
                            nc.vector.tensor_scalar_mul(
                                ach[:, ms, :], Tc[:, ms, :], c[0])
                            nc.vector.scalar_tensor_tensor(
                                ach[:, ms, :], Tn[:, ms, :], c[1],
                                ach[:, ms, :], op0=mult, op1=add)
                        else:
                            nc.vector.scalar_tensor_tensor(
                                ach[:, ms, :], Tn[:, ms, :], c[s],
                                ach[:, ms, :], op0=mult, op1=add)

                    if s < DEG:
                        nc.sync.dma_start(agin3[:], stage[:])
                        agout[s - 1][h] = dram.tile(
                            [NC * 128, MS2 * BH], f8, addr_space="Shared",
                            name=f"agout{s}_{h}")
                        nc.gpsimd.collective_compute(
                            "AllGather",
                            mybir.AluOpType.bypass,
                            replica_groups=[list(range(NC))],
                            ins=[agin[:]],
                            outs=[agout[s - 1][h][:]],
                        )

            out3 = acc_out[:, :].rearrange("(ms p) b -> p ms b", p=128)
            for h in range(NH):
                nc.sync.dma_start(out3[:, :, h * BH:(h + 1) * BH],
                                  acc[h][:])

    nc.finalize()
    return nc


def _get_program(scalars):
    key = tuple(np.asarray(scalars, np.float64).tolist())
    if key not in _BUILD_CACHE:
        _BUILD_CACHE[key] = _build(key)
    return _BUILD_CACHE[key]


def _pack_inputs(X8):
    """Pre-pack fp8 X into the DoubleRow SBUF tile layouts (contiguous
    per-partition DMA lines)."""
    # stream: [p, mp, kpair, 2, mc] <- X8[(kpair*2+i)*128+p, mp*128+mc]
    XL = np.ascontiguousarray(
        X8.reshape(KP1, 2, 128, MP1, 128).transpose(2, 3, 0, 1, 4))
    return XL


def _run(X, R, coeffs, t_mid, t_half, trace=False):
    from concourse.bass_utils import run_bass_kernel_spmd

    X = np.ascontiguousarray(np.asarray(X, np.float32))
    R = np.ascontiguousarray(np.asarray(R, np.float32))
    coeffs = np.asarray(coeffs, np.float32)
    tm = float(np.asarray(t_mid).reshape(-1)[0])
    th = float(np.asarray(t_half).reshape(-1)[0])

    nc = _get_program((tm, th, *[float(v) for v in coeffs]))

    X8 = X.astype(F8NP)
    XL = _pack_inputs(X8)
    V32 = np.ascontiguousarray(R.T.astype(np.float32))   # (N, B)
    V16p = np.ascontiguousarray(
        V32.astype(np.float16).reshape(KT2, 128, B)
        .transpose(1, 0, 2).reshape(128, KT2 * B))

    in_maps = []
    for i in range(NC):
        ib = slice(i * CB, (i + 1) * CB)
        XRi = np.ascontiguousarray(
            X8[:, ib].reshape(KP1, 2, 128, CB).transpose(2, 0, 1, 3))
        in_maps.append({
            "XL8": XL,
            "XR8": XRi,
            "V16p": V16p,
            "Vblk32": np.ascontiguousarray(V32[ib, :]),
        })

    res = run_bass_kernel_spmd(nc, in_maps, core_ids=list(range(NC)),
                               trace=trace)

    out = np.empty((B, N), np.float32)
    for i in range(NC):
        out[:, i * CB:(i + 1) * CB] = res.results[i]["acc_out"].T
    return out, res


def kernel(X, R, coeffs, t_mid, t_half):
    out, _ = _run(X, R, coeffs, t_mid, t_half, trace=False)
    return out
